# revision 15
# baseline (speedup 1.0000x reference)
"""SO3Activation Trainium2 kernel.

Math (see reference): out = einsum('bxyz,y,xyzi->bi', ACT*tanh(einsum('bi,xyzi->bxyz', f, D)/s), qw, D) * s

Sharding: the alpha (x) grid axis of D is split across the 8 cores: core c
owns x in [6c, 6c+6) -> J = 6*24*48 = 6912 grid rows, full batch B=256.
Each core produces a partial [256, 286] output (its x-slice of the
quadrature sum); the host sums the 8 partials (the unshard step).

Two device algorithms, both all-fp16 matmul operands with fp32 PSUM
accumulation (end-to-end rel err ~3.5e-4 vs the fp32 reference):

v1 (direct): per core
    Gt = Dc @ f.T                      (PE, contract 286, out [J, B] tiles)
    T  = tanh(Gt / s)                  (ACT, PSUM -> SBUF fp16)
    P  = T-tiles @ (qw-folded Dc)      (PE, contract J, accum [256, 286])

v2 (z-factored to_grid, default): D[x,y,z,:] entries are trig polynomials
of degree <=5 in the z angle, so D = Ez @ DZ exactly, with Ez [48, 11] the
trig basis on the uniform z grid and DZ 11/48 the size of D. to_grid then
becomes S1: P = f @ DZ^T (1/3 the PE rows of v1's first matmul) followed by
S2: g = Ez @ P, executed as one 128x128 block-diagonal matmul per output
tile (8 (x,y)-groups x 16 padded m' rows). tanh and the from_grid matmul
are unchanged (the second D copy is row-permuted on the host to match S2's
output ordering). PE rows/core drop 72.4k -> 58.5k; all three matmul stages
run at the fp16 PE row floor for their shapes.

Host folds qw/max(qw), ACT_CST, sqrt(dim) and max(qw) into the constants /
final scale, so the device program is input-value-agnostic.

`repeats` > 1 builds a program that executes the whole kernel body N times
back-to-back (timing harness use only; the graded path uses repeats=1).
"""

import numpy as np

B = 256
DIM = 286
NA = 48
NB = 24
NCORES = 8
XS = NA // NCORES          # alpha slices per core
J = XS * NB * NA           # 6912 grid rows per core
JT = J // 128              # 54 j-tiles per core
GRP = 6                    # j-tiles per DMA/const group
CH = JT // GRP             # 9 groups
KS = [128, 128, 30]        # K tiling of DIM=286
S = float(np.sqrt(np.float32(DIM)))
ACT_CST = 1.5925374197228315

_NC_CACHE = {}

# v2: z-factored to_grid. D[x,y,z,:] entries are trig polys of degree <=5 in
# the z angle, so D = Ez @ DZ with Ez [48, 11] and DZ 48/11 the size of D.
# to_grid then becomes S1: P = f @ DZ^T (small) followed by S2: g = Ez @ P
# (block-diagonal matmul, 8 (x,y)-groups of 16 padded m' rows per PE tile).
# from_grid stays direct; its D copy is row-permuted on the host to match
# S2's output ordering. PE rows drop from 72.4k to 58.5k per core.
MP = 16                     # m' padded 11 -> 16 (8 groups tile 128 exactly)
NM = 11                     # true z-basis size (1, cos/sin 1..5)
VT = XS * NB * MP // 128    # S1 output tiles per core = 18
ZC = NA // MP               # z chunks of 16 = 3
VU = VT * ZC                # 54 S2 units (== v1 j-tiles)


def build_nc(repeats=1, load_each_repeat=True):
    import concourse.mybir as mybir
    import concourse.tile as tile
    from concourse import bacc

    f16 = mybir.dt.float16
    f32 = mybir.dt.float32

    nc = bacc.Bacc()
    if repeats != 1 or not load_each_repeat:
        # structurally unique input per variant: the PJRT-side executable
        # cache fingerprints the HLO without the custom-call payload, so
        # same-shaped variants would collide and reuse the wrong NEFF
        nc.declare_dram_parameter(
            "pad", [1, 2 * repeats + int(load_each_repeat)], f16, isOutput=False
        )
    dtt = nc.declare_dram_parameter("dtt", [DIM, J], f16, isOutput=False)
    dcw = nc.declare_dram_parameter("dcw", [J, DIM], f16, isOutput=False)
    ft = nc.declare_dram_parameter("ft", [DIM, B], f16, isOutput=False)
    out = nc.declare_dram_parameter("out", [B, DIM], f32, isOutput=True)
    tanh = mybir.ActivationFunctionType.Tanh

    with tile.TileContext(nc) as tc:
        with (
            tc.tile_pool(name="const", bufs=1) as cpool,
            tc.tile_pool(name="gps", bufs=3, space="PSUM") as gpool,
            tc.tile_pool(name="ops", bufs=1, space="PSUM") as opool,
            tc.tile_pool(name="work", bufs=3) as wpool,
            tc.tile_pool(name="osb", bufs=1) as obpool,
        ):
            ftt = [
                cpool.tile([kk, B], f16, tag=f"ft{k}", name=f"ft{k}")
                for k, kk in enumerate(KS)
            ]
            dtt_t = [
                [
                    cpool.tile(
                        [kk, GRP * 128], f16, tag=f"dtt{c}_{k}", name=f"dtt{c}_{k}"
                    )
                    for k, kk in enumerate(KS)
                ]
                for c in range(CH)
            ]
            dcw_t = [
                cpool.tile([128, GRP, DIM], f16, tag=f"dcw{c}", name=f"dcw{c}")
                for c in range(CH)
            ]
            op = [
                opool.tile([128, DIM], f32, tag=f"outp{h}", name=f"outp{h}")
                for h in range(2)
            ]

            def load_consts():
                ko = 0
                for k, kk in enumerate(KS):
                    nc.sync.dma_start(out=ftt[k][:], in_=ft[ko : ko + kk, :])
                    ko += kk
                for c in range(CH):
                    ko = 0
                    for k, kk in enumerate(KS):
                        nc.sync.dma_start(
                            out=dtt_t[c][k][:],
                            in_=dtt[ko : ko + kk, c * GRP * 128 : (c + 1) * GRP * 128],
                        )
                        ko += kk
                    src = dcw[c * GRP * 128 : (c + 1) * GRP * 128, :].rearrange(
                        "(sub p) i -> p sub i", p=128
                    )
                    nc.sync.dma_start(out=dcw_t[c][:], in_=src)

            for rep in range(repeats):
                if rep == 0 or load_each_repeat:
                    load_consts()

                def emit_mm2(pair, tt):
                    for ti, t in enumerate((2 * pair, 2 * pair + 1)):
                        c, sub = divmod(t, GRP)
                        rhs = dcw_t[c][:, sub, :]
                        for h in range(2):
                            nc.tensor.matmul(
                                op[h],
                                tt[:, ti * 256 + h * 128 : ti * 256 + (h + 1) * 128],
                                rhs,
                                start=(t == 0),
                                stop=(t == JT - 1),
                                skip_group_check=True,
                            )

                # software pipeline: emit pair p's MM2 after pair p+1's MM1 so
                # the PE hides the tanh latency behind the next pair's MM1
                pending = None
                for pair in range(JT // 2):
                    g = gpool.tile([128, 512], f32, tag="g", name=f"g{rep}_{pair}")
                    # per-pair tag: slot reuse within a repeat would add a
                    # second sync-wait to the tanh (1-wait ISA limit)
                    tt = wpool.tile(
                        [128, 512], f16, tag=f"tt{pair}", name=f"tt{rep}_{pair}"
                    )
                    for ti, t in enumerate((2 * pair, 2 * pair + 1)):
                        c, sub = divmod(t, GRP)
                        for k in range(len(KS)):
                            nc.tensor.matmul(
                                g[:, ti * 256 : (ti + 1) * 256],
                                dtt_t[c][k][:, sub * 128 : (sub + 1) * 128],
                                ftt[k],
                                start=(k == 0),
                                stop=(k == len(KS) - 1),
                            )
                    nc.scalar.activation(tt[:], g[:], tanh, scale=1.0 / S)
                    if pending is not None:
                        emit_mm2(*pending)
                    pending = (pair, tt)
                emit_mm2(*pending)

                for h in range(2):
                    ob = obpool.tile(
                        [128, DIM], f32, tag=f"ob{h}", name=f"ob{rep}_{h}"
                    )
                    nc.vector.tensor_copy(ob[:], op[h][:])
                    # gpsimd queue: the sync/SP queue's ring-slot wait would be
                    # a second sync-wait on this DMA (1-wait ISA limit)
                    nc.gpsimd.dma_start(
                        out=out[h * 128 : (h + 1) * 128, :], in_=ob[:]
                    )
    nc.finalize()
    return nc


def build_nc_v2(repeats=1, load_each_repeat=True):
    import concourse.mybir as mybir
    import concourse.tile as tile
    from concourse import bacc

    f16 = mybir.dt.float16
    f32 = mybir.dt.float32

    nc = bacc.Bacc()
    if repeats != 1 or not load_each_repeat:
        nc.declare_dram_parameter(
            "pad", [2, 2 * repeats + int(load_each_repeat)], f16, isOutput=False
        )
    wt = nc.declare_dram_parameter("wt", [DIM, VT * 128], f16, isOutput=False)
    eb = nc.declare_dram_parameter("eb", [ZC, 128, 128], f16, isOutput=False)
    dcw = nc.declare_dram_parameter("dcw", [VU * 128, DIM], f16, isOutput=False)
    ft = nc.declare_dram_parameter("ft", [DIM, B], f16, isOutput=False)
    out = nc.declare_dram_parameter("out", [B, DIM], f32, isOutput=True)
    tanh = mybir.ActivationFunctionType.Tanh

    WCH = VT // GRP  # wt chunks of 6 tiles = 3

    with tile.TileContext(nc) as tc:
        with (
            tc.tile_pool(name="const", bufs=1) as cpool,
            tc.tile_pool(name="p1ps", bufs=2, space="PSUM") as p1pool,
            tc.tile_pool(name="gps", bufs=3, space="PSUM") as gpool,
            tc.tile_pool(name="ops", bufs=1, space="PSUM") as opool,
            tc.tile_pool(name="ptsb", bufs=1) as ptpool,
            tc.tile_pool(name="work", bufs=3) as wpool,
            tc.tile_pool(name="osb", bufs=1) as obpool,
        ):
            ftt = [
                cpool.tile([kk, B], f16, tag=f"ft{k}", name=f"ft{k}")
                for k, kk in enumerate(KS)
            ]
            wt_t = [
                [
                    cpool.tile([kk, GRP * 128], f16, tag=f"wt{c}_{k}", name=f"wt{c}_{k}")
                    for k, kk in enumerate(KS)
                ]
                for c in range(WCH)
            ]
            eb_t = [
                cpool.tile([128, 128], f16, tag=f"eb{c}", name=f"eb{c}")
                for c in range(ZC)
            ]
            dcw_t = [
                cpool.tile([128, GRP, DIM], f16, tag=f"dcw{c}", name=f"dcw{c}")
                for c in range(CH)
            ]
            op = [
                opool.tile([128, DIM], f32, tag=f"outp{h}", name=f"outp{h}")
                for h in range(2)
            ]

            def load_consts():
                ko = 0
                for k, kk in enumerate(KS):
                    nc.sync.dma_start(out=ftt[k][:], in_=ft[ko : ko + kk, :])
                    ko += kk
                for c in range(ZC):
                    nc.sync.dma_start(out=eb_t[c][:], in_=eb[c])
                for c in range(WCH):
                    ko = 0
                    for k, kk in enumerate(KS):
                        nc.sync.dma_start(
                            out=wt_t[c][k][:],
                            in_=wt[ko : ko + kk, c * GRP * 128 : (c + 1) * GRP * 128],
                        )
                        ko += kk
                for c in range(CH):
                    src = dcw[c * GRP * 128 : (c + 1) * GRP * 128, :].rearrange(
                        "(sub p) i -> p sub i", p=128
                    )
                    nc.sync.dma_start(out=dcw_t[c][:], in_=src)

            for rep in range(repeats):
                if rep == 0 or load_each_repeat:
                    load_consts()

                def emit_s1(T):
                    ps = p1pool.tile([128, B], f32, tag="p1", name=f"p1_{rep}_{T}")
                    c, sub = divmod(T, GRP)
                    for k in range(len(KS)):
                        nc.tensor.matmul(
                            ps[:],
                            wt_t[c][k][:, sub * 128 : (sub + 1) * 128],
                            ftt[k],
                            start=(k == 0),
                            stop=(k == len(KS) - 1),
                        )
                    pt = ptpool.tile(
                        [128, B], f16, tag=f"pt{T % 4}", name=f"pt{rep}_{T}"
                    )
                    nc.vector.tensor_copy(pt[:], ps[:])
                    return pt

                def emit_mm2(pair, tt):
                    for ti in range(2):
                        u = 2 * pair + ti
                        c, sub = divmod(u, GRP)
                        rhs = dcw_t[c][:, sub, :]
                        for h in range(2):
                            nc.tensor.matmul(
                                op[h],
                                tt[:, ti * 256 + h * 128 : ti * 256 + (h + 1) * 128],
                                rhs,
                                start=(u == 0),
                                stop=(u == VU - 1),
                                skip_group_check=True,
                            )

                pt_tiles = {}
                pt_tiles[0] = emit_s1(0)
                g = None
                tt = None
                pending = None
                for u in range(VU):
                    T, zc = divmod(u, ZC)
                    if T + 1 in range(VT) and T + 1 not in pt_tiles and zc == 0:
                        pt_tiles[T + 1] = emit_s1(T + 1)
                    half = u % 2
                    if half == 0:
                        g = gpool.tile([128, 512], f32, tag="g", name=f"g{rep}_{u}")
                    nc.tensor.matmul(
                        g[:, half * 256 : (half + 1) * 256],
                        eb_t[zc][:],
                        pt_tiles[T][:],
                        start=True,
                        stop=True,
                    )
                    if half == 1:
                        pair = u // 2
                        tt = wpool.tile(
                            [128, 512], f16, tag=f"tt{pair}", name=f"tt{rep}_{pair}"
                        )
                        nc.scalar.activation(tt[:], g[:], tanh, scale=1.0 / S)
                        if pending is not None:
                            emit_mm2(*pending)
                        pending = (pair, tt)
                emit_mm2(*pending)

                for h in range(2):
                    ob = obpool.tile(
                        [128, DIM], f32, tag=f"ob{h}", name=f"ob{rep}_{h}"
                    )
                    nc.vector.tensor_copy(ob[:], op[h][:])
                    nc.gpsimd.dma_start(
                        out=out[h * 128 : (h + 1) * 128, :], in_=ob[:]
                    )
    nc.finalize()
    return nc


def get_nc(repeats=1, load_each_repeat=True, algo="v1"):
    key = (repeats, load_each_repeat, algo)
    if key not in _NC_CACHE:
        builder = build_nc if algo == "v1" else build_nc_v2
        _NC_CACHE[key] = builder(repeats, load_each_repeat)
    return _NC_CACHE[key]


def make_in_maps(features, D, qw):
    features = np.asarray(features, dtype=np.float32)
    D = np.asarray(D, dtype=np.float32)
    qw = np.asarray(qw, dtype=np.float32)

    qm = float(qw.max())
    qcol = (np.repeat(np.tile(qw, XS), NA) / qm).astype(np.float32)  # [J]
    ft_np = np.ascontiguousarray(features.T).astype(np.float16)

    in_maps = []
    for c in range(NCORES):
        dc = D[XS * c : XS * (c + 1)].reshape(J, DIM)
        in_maps.append(
            {
                "dtt": np.ascontiguousarray(dc.T).astype(np.float16),
                "dcw": (dc * qcol[:, None]).astype(np.float16),
                "ft": ft_np,
            }
        )
    return in_maps, qm


def make_in_maps_v2(features, D, qw):
    features = np.asarray(features, dtype=np.float32)
    D = np.asarray(D, dtype=np.float32)
    qw = np.asarray(qw, dtype=np.float32)

    qm = float(qw.max())
    qcol = (np.repeat(np.tile(qw, XS), NA) / qm).astype(np.float32)  # [J]
    ft_np = np.ascontiguousarray(features.T).astype(np.float16)

    ang = np.arange(NA) / NA * 2.0 * np.pi
    cols = [np.ones(NA)]
    for m in range(1, (NM - 1) // 2 + 1):
        cols += [np.cos(m * ang), np.sin(m * ang)]
    ez = np.stack(cols, axis=1)  # [48, 11] float64
    ez_pinv = np.linalg.pinv(ez)  # [11, 48]

    eb = np.zeros((ZC, 128, 128), np.float16)
    for zc in range(ZC):
        for g in range(128 // MP):
            blk = ez[zc * MP : (zc + 1) * MP, :].T  # [11, 16]
            eb[zc, g * MP : g * MP + NM, g * MP : (g + 1) * MP] = blk

    # dcw row permutation: unit u=(T,zc), partition p=(g,zl) -> grid row j
    u = np.arange(VU * 128)
    uu, p = u // 128, u % 128
    T, zc = uu // ZC, uu % ZC
    g, zl = p // MP, p % MP
    G = T * (128 // MP) + g
    perm = G * NA + zc * MP + zl

    in_maps = []
    for c in range(NCORES):
        dsl = D[XS * c : XS * (c + 1)]  # [6, 24, 48, 286]
        dz = np.einsum("mz,xyzi->xymi", ez_pinv, dsl.astype(np.float64))
        wc = np.zeros((XS, NB, MP, DIM), np.float32)
        wc[:, :, :NM, :] = dz
        wt_np = np.ascontiguousarray(wc.reshape(-1, DIM).T).astype(np.float16)
        dc = dsl.reshape(J, DIM)
        dcw_np = (dc * qcol[:, None]).astype(np.float16)[perm]
        in_maps.append(
            {"wt": wt_np, "eb": eb, "dcw": dcw_np, "ft": ft_np}
        )
    return in_maps, qm


def _run(features, D, qw, algo):
    from concourse.bass_utils import run_bass_kernel_spmd

    maker = make_in_maps_v2 if algo == "v2" else make_in_maps
    in_maps, qm = maker(features, D, qw)
    nc = get_nc(algo=algo)
    res = run_bass_kernel_spmd(nc, in_maps, list(range(NCORES)))
    kernel.last_results = res

    total = np.zeros((B, DIM), dtype=np.float64)
    for r in res.results:
        total += r["out"].astype(np.float64)
    return (total * (ACT_CST * S * qm)).astype(np.float32)


def kernel(features, D, qw, algo="v2"):
    if algo == "v2":
        try:
            return _run(features, D, qw, "v2")
        except Exception:
            pass  # fall back to the direct algorithm
    return _run(features, D, qw, "v1")


# revision 21
# speedup vs baseline: 1.0624x; 1.0624x over previous
"""SO3Activation Trainium2 kernel.

Math (see reference): out = einsum('bxyz,y,xyzi->bi', ACT*tanh(einsum('bi,xyzi->bxyz', f, D)/s), qw, D) * s

Sharding: the alpha (x) grid axis of D is split across the 8 cores: core c
owns x in [6c, 6c+6) -> J = 6*24*48 = 6912 grid rows, full batch B=256.
Each core produces a partial [256, 286] output (its x-slice of the
quadrature sum); the host sums the 8 partials (the unshard step).

Two device algorithms, both all-fp16 matmul operands with fp32 PSUM
accumulation (end-to-end rel err ~3.5e-4 vs the fp32 reference):

v1 (direct): per core
    Gt = Dc @ f.T                      (PE, contract 286, out [J, B] tiles)
    T  = tanh(Gt / s)                  (ACT, PSUM -> SBUF fp16)
    P  = T-tiles @ (qw-folded Dc)      (PE, contract J, accum [256, 286])

v2 (z-factored to_grid, default): D[x,y,z,:] entries are trig polynomials
of degree <=5 in the z angle, so D = Ez @ DZ exactly, with Ez [48, 11] the
trig basis on the uniform z grid and DZ 11/48 the size of D. to_grid then
becomes S1: P = f @ DZ^T (1/3 the PE rows of v1's first matmul) followed by
S2: g = Ez @ P, executed as one 128x128 block-diagonal matmul per output
tile (8 (x,y)-groups x 16 padded m' rows). tanh and the from_grid matmul
are unchanged (the second D copy is row-permuted on the host to match S2's
output ordering). PE rows/core drop 72.4k -> 58.5k; all three matmul stages
run at the fp16 PE row floor for their shapes.

Host folds qw/max(qw), ACT_CST, sqrt(dim) and max(qw) into the constants /
final scale, so the device program is input-value-agnostic.

`repeats` > 1 builds a program that executes the whole kernel body N times
back-to-back (timing harness use only; the graded path uses repeats=1).
"""

import numpy as np

B = 256
DIM = 286
NA = 48
NB = 24
NCORES = 8
XS = NA // NCORES          # alpha slices per core
J = XS * NB * NA           # 6912 grid rows per core
JT = J // 128              # 54 j-tiles per core
GRP = 6                    # j-tiles per DMA/const group
CH = JT // GRP             # 9 groups
KS = [128, 128, 30]        # K tiling of DIM=286
S = float(np.sqrt(np.float32(DIM)))
ACT_CST = 1.5925374197228315

_NC_CACHE = {}

# v2: z-factored to_grid. D[x,y,z,:] entries are trig polys of degree <=5 in
# the z angle, so D = Ez @ DZ with Ez [48, 11] and DZ 48/11 the size of D.
# to_grid then becomes S1: P = f @ DZ^T (small) followed by S2: g = Ez @ P
# (block-diagonal matmul, 8 (x,y)-groups of 16 padded m' rows per PE tile).
# from_grid stays direct; its D copy is row-permuted on the host to match
# S2's output ordering. PE rows drop from 72.4k to 58.5k per core.
MP = 16                     # m' padded 11 -> 16 (8 groups tile 128 exactly)
NM = 11                     # true z-basis size (1, cos/sin 1..5)
VT = XS * NB * MP // 128    # S1 output tiles per core = 18
ZC = NA // MP               # z chunks of 16 = 3
VU = VT * ZC                # 54 S2 units (== v1 j-tiles)


def build_nc(repeats=1, load_each_repeat=True):
    import concourse.mybir as mybir
    import concourse.tile as tile
    from concourse import bacc

    f16 = mybir.dt.float16
    f32 = mybir.dt.float32

    nc = bacc.Bacc()
    if repeats != 1 or not load_each_repeat:
        # structurally unique input per variant: the PJRT-side executable
        # cache fingerprints the HLO without the custom-call payload, so
        # same-shaped variants would collide and reuse the wrong NEFF
        nc.declare_dram_parameter(
            "pad", [1, 2 * repeats + int(load_each_repeat)], f16, isOutput=False
        )
    dtt = nc.declare_dram_parameter("dtt", [DIM, J], f16, isOutput=False)
    dcw = nc.declare_dram_parameter("dcw", [J, DIM], f16, isOutput=False)
    ft = nc.declare_dram_parameter("ft", [DIM, B], f16, isOutput=False)
    out = nc.declare_dram_parameter("out", [B, DIM], f32, isOutput=True)
    tanh = mybir.ActivationFunctionType.Tanh

    with tile.TileContext(nc) as tc:
        with (
            tc.tile_pool(name="const", bufs=1) as cpool,
            tc.tile_pool(name="gps", bufs=3, space="PSUM") as gpool,
            tc.tile_pool(name="ops", bufs=1, space="PSUM") as opool,
            tc.tile_pool(name="work", bufs=3) as wpool,
            tc.tile_pool(name="osb", bufs=1) as obpool,
        ):
            ftt = [
                cpool.tile([kk, B], f16, tag=f"ft{k}", name=f"ft{k}")
                for k, kk in enumerate(KS)
            ]
            dtt_t = [
                [
                    cpool.tile(
                        [kk, GRP * 128], f16, tag=f"dtt{c}_{k}", name=f"dtt{c}_{k}"
                    )
                    for k, kk in enumerate(KS)
                ]
                for c in range(CH)
            ]
            dcw_t = [
                cpool.tile([128, GRP, DIM], f16, tag=f"dcw{c}", name=f"dcw{c}")
                for c in range(CH)
            ]
            op = [
                opool.tile([128, DIM], f32, tag=f"outp{h}", name=f"outp{h}")
                for h in range(2)
            ]

            def load_consts():
                ko = 0
                for k, kk in enumerate(KS):
                    nc.sync.dma_start(out=ftt[k][:], in_=ft[ko : ko + kk, :])
                    ko += kk
                for c in range(CH):
                    ko = 0
                    for k, kk in enumerate(KS):
                        nc.sync.dma_start(
                            out=dtt_t[c][k][:],
                            in_=dtt[ko : ko + kk, c * GRP * 128 : (c + 1) * GRP * 128],
                        )
                        ko += kk
                    src = dcw[c * GRP * 128 : (c + 1) * GRP * 128, :].rearrange(
                        "(sub p) i -> p sub i", p=128
                    )
                    nc.sync.dma_start(out=dcw_t[c][:], in_=src)

            for rep in range(repeats):
                if rep == 0 or load_each_repeat:
                    load_consts()

                def emit_mm2(pair, tt):
                    for ti, t in enumerate((2 * pair, 2 * pair + 1)):
                        c, sub = divmod(t, GRP)
                        rhs = dcw_t[c][:, sub, :]
                        for h in range(2):
                            nc.tensor.matmul(
                                op[h],
                                tt[:, ti * 256 + h * 128 : ti * 256 + (h + 1) * 128],
                                rhs,
                                start=(t == 0),
                                stop=(t == JT - 1),
                                skip_group_check=True,
                            )

                # software pipeline: emit pair p's MM2 after pair p+1's MM1 so
                # the PE hides the tanh latency behind the next pair's MM1
                pending = None
                for pair in range(JT // 2):
                    g = gpool.tile([128, 512], f32, tag="g", name=f"g{rep}_{pair}")
                    # per-pair tag: slot reuse within a repeat would add a
                    # second sync-wait to the tanh (1-wait ISA limit)
                    tt = wpool.tile(
                        [128, 512], f16, tag=f"tt{pair}", name=f"tt{rep}_{pair}"
                    )
                    for ti, t in enumerate((2 * pair, 2 * pair + 1)):
                        c, sub = divmod(t, GRP)
                        for k in range(len(KS)):
                            nc.tensor.matmul(
                                g[:, ti * 256 : (ti + 1) * 256],
                                dtt_t[c][k][:, sub * 128 : (sub + 1) * 128],
                                ftt[k],
                                start=(k == 0),
                                stop=(k == len(KS) - 1),
                            )
                    nc.scalar.activation(tt[:], g[:], tanh, scale=1.0 / S)
                    if pending is not None:
                        emit_mm2(*pending)
                    pending = (pair, tt)
                emit_mm2(*pending)

                for h in range(2):
                    ob = obpool.tile(
                        [128, DIM], f32, tag=f"ob{h}", name=f"ob{rep}_{h}"
                    )
                    nc.vector.tensor_copy(ob[:], op[h][:])
                    # gpsimd queue: the sync/SP queue's ring-slot wait would be
                    # a second sync-wait on this DMA (1-wait ISA limit)
                    nc.gpsimd.dma_start(
                        out=out[h * 128 : (h + 1) * 128, :], in_=ob[:]
                    )
    nc.finalize()
    return nc


def build_nc_v2(repeats=1, load_each_repeat=True):
    import concourse.mybir as mybir
    import concourse.tile as tile
    from concourse import bacc

    f16 = mybir.dt.float16
    f32 = mybir.dt.float32

    nc = bacc.Bacc()
    if repeats != 1 or not load_each_repeat:
        nc.declare_dram_parameter(
            "pad", [2, 2 * repeats + int(load_each_repeat)], f16, isOutput=False
        )
    wt = nc.declare_dram_parameter("wt", [DIM, VT * 128], f16, isOutput=False)
    eb = nc.declare_dram_parameter("eb", [ZC, 128, 128], f16, isOutput=False)
    dcw = nc.declare_dram_parameter("dcw", [VU * 128, DIM], f16, isOutput=False)
    ft = nc.declare_dram_parameter("ft", [DIM, B], f16, isOutput=False)
    out = nc.declare_dram_parameter("out", [B, DIM], f32, isOutput=True)
    tanh = mybir.ActivationFunctionType.Tanh

    WCH = VT // GRP  # wt chunks of 6 tiles = 3

    with tile.TileContext(nc) as tc:
        with (
            tc.tile_pool(name="const", bufs=1) as cpool,
            tc.tile_pool(name="p1ps", bufs=2, space="PSUM") as p1pool,
            tc.tile_pool(name="gps", bufs=3, space="PSUM") as gpool,
            tc.tile_pool(name="ops", bufs=1, space="PSUM") as opool,
            tc.tile_pool(name="ptsb", bufs=1) as ptpool,
            tc.tile_pool(name="work", bufs=3) as wpool,
            tc.tile_pool(name="osb", bufs=1) as obpool,
        ):
            ftt = [
                cpool.tile([kk, B], f16, tag=f"ft{k}", name=f"ft{k}")
                for k, kk in enumerate(KS)
            ]
            wt_t = [
                [
                    cpool.tile([kk, GRP * 128], f16, tag=f"wt{c}_{k}", name=f"wt{c}_{k}")
                    for k, kk in enumerate(KS)
                ]
                for c in range(WCH)
            ]
            eb_t = [
                cpool.tile([128, 128], f16, tag=f"eb{c}", name=f"eb{c}")
                for c in range(ZC)
            ]
            dcw_t = [
                cpool.tile([128, GRP, DIM], f16, tag=f"dcw{c}", name=f"dcw{c}")
                for c in range(CH)
            ]
            op = [
                opool.tile([128, DIM], f32, tag=f"outp{h}", name=f"outp{h}")
                for h in range(2)
            ]

            def load_consts(first=False):
                # chunk 0 first, ft/wt interleaved per k-tile, so S1(0)'s
                # k-th matmul can start as soon as its two operands land
                ko = 0
                for k, kk in enumerate(KS):
                    nc.sync.dma_start(out=ftt[k][:], in_=ft[ko : ko + kk, :])
                    nc.sync.dma_start(
                        out=wt_t[0][k][:], in_=wt[ko : ko + kk, 0 : GRP * 128]
                    )
                    ko += kk
                for c in range(WCH):
                    if c == 0:
                        # eb after wt chunk 0: S1(0) starts sooner; first S2
                        # only needs eb after S1(0) completes
                        for z in range(ZC):
                            nc.sync.dma_start(out=eb_t[z][:], in_=eb[z])
                        continue
                    ko = 0
                    for k, kk in enumerate(KS):
                        nc.sync.dma_start(
                            out=wt_t[c][k][:],
                            in_=wt[ko : ko + kk, c * GRP * 128 : (c + 1) * GRP * 128],
                        )
                        ko += kk
                for c in range(CH):
                    src = dcw[c * GRP * 128 : (c + 1) * GRP * 128, :].rearrange(
                        "(sub p) i -> p sub i", p=128
                    )
                    nc.sync.dma_start(out=dcw_t[c][:], in_=src)

            # HAM warmup: the PE clock sits at 4/8 throttle until ~3.4us of
            # sustained activity. Dummy matmuls during the DMA fill window
            # bring the real matmuls up at full clock.
            with tc.tile_pool(name="wups", bufs=1, space="PSUM") as wupool:
                wu = cpool.tile([16, 512], f16, tag="wu", name="wu")
                nc.any.memzero(wu)
                wups = wupool.tile([16, 512], f32, tag="wup", name="wup")
                for i in range(6):
                    nc.tensor.matmul(
                        wups[:], wu[:, :16], wu[:], start=True, stop=True
                    )

            for rep in range(repeats):
                if rep == 0 or load_each_repeat:
                    load_consts()

                def emit_s1(T):
                    ps = p1pool.tile([128, B], f32, tag="p1", name=f"p1_{rep}_{T}")
                    c, sub = divmod(T, GRP)
                    for k in range(len(KS)):
                        nc.tensor.matmul(
                            ps[:],
                            wt_t[c][k][:, sub * 128 : (sub + 1) * 128],
                            ftt[k],
                            start=(k == 0),
                            stop=(k == len(KS) - 1),
                        )
                    pt = ptpool.tile(
                        [128, B], f16, tag=f"pt{T % 4}", name=f"pt{rep}_{T}"
                    )
                    nc.vector.tensor_copy(pt[:], ps[:])
                    return pt

                def emit_mm2(pair, tt):
                    for ti in range(2):
                        u = 2 * pair + ti
                        c, sub = divmod(u, GRP)
                        rhs = dcw_t[c][:, sub, :]
                        for h in range(2):
                            nc.tensor.matmul(
                                op[h],
                                tt[:, ti * 256 + h * 128 : ti * 256 + (h + 1) * 128],
                                rhs,
                                start=(u == 0),
                                stop=(u == VU - 1),
                                skip_group_check=True,
                            )

                pt_tiles = {}
                pt_tiles[0] = emit_s1(0)
                g = None
                tt = None
                # delay each pair's MM2 two pairs behind its tanh so the PE
                # hides the ACT latency behind later S1/S2 work
                pending = []
                for u in range(VU):
                    T, zc = divmod(u, ZC)
                    if T + 1 in range(VT) and T + 1 not in pt_tiles and zc == 0:
                        pt_tiles[T + 1] = emit_s1(T + 1)
                    half = u % 2
                    if half == 0:
                        g = gpool.tile([128, 512], f32, tag="g", name=f"g{rep}_{u}")
                    nc.tensor.matmul(
                        g[:, half * 256 : (half + 1) * 256],
                        eb_t[zc][:],
                        pt_tiles[T][:],
                        start=True,
                        stop=True,
                    )
                    if half == 1:
                        pair = u // 2
                        tt = wpool.tile(
                            [128, 512], f16, tag=f"tt{pair}", name=f"tt{rep}_{pair}"
                        )
                        nc.scalar.activation(tt[:], g[:], tanh, scale=1.0 / S)
                        pending.append((pair, tt))
                        if len(pending) > 2:
                            emit_mm2(*pending.pop(0))
                for p in pending:
                    emit_mm2(*p)

                for h in range(2):
                    ob = obpool.tile(
                        [128, DIM], f32, tag=f"ob{h}", name=f"ob{rep}_{h}"
                    )
                    nc.vector.tensor_copy(ob[:], op[h][:])
                    nc.gpsimd.dma_start(
                        out=out[h * 128 : (h + 1) * 128, :], in_=ob[:]
                    )
    nc.finalize()
    return nc


def get_nc(repeats=1, load_each_repeat=True, algo="v1"):
    key = (repeats, load_each_repeat, algo)
    if key not in _NC_CACHE:
        builder = build_nc if algo == "v1" else build_nc_v2
        _NC_CACHE[key] = builder(repeats, load_each_repeat)
    return _NC_CACHE[key]


def make_in_maps(features, D, qw):
    features = np.asarray(features, dtype=np.float32)
    D = np.asarray(D, dtype=np.float32)
    qw = np.asarray(qw, dtype=np.float32)

    qm = float(qw.max())
    qcol = (np.repeat(np.tile(qw, XS), NA) / qm).astype(np.float32)  # [J]
    ft_np = np.ascontiguousarray(features.T).astype(np.float16)

    in_maps = []
    for c in range(NCORES):
        dc = D[XS * c : XS * (c + 1)].reshape(J, DIM)
        in_maps.append(
            {
                "dtt": np.ascontiguousarray(dc.T).astype(np.float16),
                "dcw": (dc * qcol[:, None]).astype(np.float16),
                "ft": ft_np,
            }
        )
    return in_maps, qm


def make_in_maps_v2(features, D, qw):
    features = np.asarray(features, dtype=np.float32)
    D = np.asarray(D, dtype=np.float32)
    qw = np.asarray(qw, dtype=np.float32)

    qm = float(qw.max())
    qcol = (np.repeat(np.tile(qw, XS), NA) / qm).astype(np.float32)  # [J]
    ft_np = np.ascontiguousarray(features.T).astype(np.float16)

    ang = np.arange(NA) / NA * 2.0 * np.pi
    cols = [np.ones(NA)]
    for m in range(1, (NM - 1) // 2 + 1):
        cols += [np.cos(m * ang), np.sin(m * ang)]
    ez = np.stack(cols, axis=1)  # [48, 11] float64
    ez_pinv = np.linalg.pinv(ez)  # [11, 48]

    eb = np.zeros((ZC, 128, 128), np.float16)
    for zc in range(ZC):
        for g in range(128 // MP):
            blk = ez[zc * MP : (zc + 1) * MP, :].T  # [11, 16]
            eb[zc, g * MP : g * MP + NM, g * MP : (g + 1) * MP] = blk

    # dcw row permutation: unit u=(T,zc), partition p=(g,zl) -> grid row j
    u = np.arange(VU * 128)
    uu, p = u // 128, u % 128
    T, zc = uu // ZC, uu % ZC
    g, zl = p // MP, p % MP
    G = T * (128 // MP) + g
    perm = G * NA + zc * MP + zl

    in_maps = []
    for c in range(NCORES):
        dsl = D[XS * c : XS * (c + 1)]  # [6, 24, 48, 286]
        dz = np.einsum("mz,xyzi->xymi", ez_pinv, dsl.astype(np.float64))
        wc = np.zeros((XS, NB, MP, DIM), np.float32)
        wc[:, :, :NM, :] = dz
        wt_np = np.ascontiguousarray(wc.reshape(-1, DIM).T).astype(np.float16)
        dc = dsl.reshape(J, DIM)
        dcw_np = (dc * qcol[:, None]).astype(np.float16)[perm]
        in_maps.append(
            {"wt": wt_np, "eb": eb, "dcw": dcw_np, "ft": ft_np}
        )
    return in_maps, qm


def _run(features, D, qw, algo):
    from concourse.bass_utils import run_bass_kernel_spmd

    maker = make_in_maps_v2 if algo == "v2" else make_in_maps
    in_maps, qm = maker(features, D, qw)
    nc = get_nc(algo=algo)
    res = run_bass_kernel_spmd(nc, in_maps, list(range(NCORES)))
    kernel.last_results = res

    total = np.zeros((B, DIM), dtype=np.float64)
    for r in res.results:
        total += r["out"].astype(np.float64)
    return (total * (ACT_CST * S * qm)).astype(np.float32)


def kernel(features, D, qw, algo="v2"):
    if algo == "v2":
        try:
            return _run(features, D, qw, "v2")
        except Exception:
            pass  # fall back to the direct algorithm
    return _run(features, D, qw, "v1")
